# revision 5
# baseline (speedup 1.0000x reference)
"""Trainium2 distributed kernel for the FM/GNN rating model (nn_APM_16733192585590).

Math: rate = x@w_lin + 0.5*sum((xV)^2 - x^2 V^2) + bias_u[uid] + bias_i[iid] + 3
with x = [user_table[uid], word_table[uid], item_table[iid], word_table[iid+NU]].

Decomposition (x = [u | i], V = [V_U ; V_I], G_ui = V_U @ V_I.T):
  rate_b = alpha_u[uid_b] + alpha_i[iid_b] + U_emb[uid_b] @ G_ui @ I_emb[iid_b]
where alpha_* fold the row-local linear + quadratic + bias terms (+1.5 each).
Precomputing Uhat = U_emb @ G_ui per table row turns the device work into
two row gathers and a row-wise dot:  rate = dot(Uhat[uid], I_emb[iid]) + alphas.

Device (per core, batch shard of 2048, mapped b -> (partition b//16, tile b%16)):
  - 2x16 indirect row gathers (544B rows: [128 floats | alpha | pad])
  - 16 fused tensor_tensor_reduce (dot + alpha_u as reduce-init)
  - 1 tensor_add (alpha_i), 1 contiguous output DMA. No transposes, no matmuls.
"""

import numpy as np

from concourse import bacc, bass, mybir
import concourse.tile as tile
from concourse.bass_utils import run_bass_kernel_spmd

N_USERS = 100000
N_ITEMS = 100000
DIM = 64
EMB = 2 * DIM          # 128 combined embedding floats per row
R = 136                # padded row length (544B): [emb(128), alpha(1), pad(7)]
BATCH = 16384
N_CORES = 8
SHARD = BATCH // N_CORES      # 2048
P = 128
T = SHARD // P                # 16 tiles of 128 batch elements

_nc_cache = {}


def _build_nc(finalize=True):
    if finalize and "nc" in _nc_cache:
        return _nc_cache["nc"]
    f32 = mybir.dt.float32
    i32 = mybir.dt.int32

    nc = bacc.Bacc(None, target_bir_lowering=False, debug=False)
    uidx = nc.declare_dram_parameter("uidx", [P, T], i32, isOutput=False)
    iidx = nc.declare_dram_parameter("iidx", [P, T], i32, isOutput=False)
    utab = nc.declare_dram_parameter("utab", [N_USERS, R], f32, isOutput=False)
    itab = nc.declare_dram_parameter("itab", [N_ITEMS, R], f32, isOutput=False)
    out = nc.declare_dram_parameter("out", [P, T], f32, isOutput=True)

    with tile.TileContext(nc) as tc:
        with tc.tile_pool(name="p", bufs=1) as pool:
            uix = pool.tile([P, T], i32)
            iix = pool.tile([P, T], i32)
            nc.sync.dma_start(uix[:], uidx[:])
            nc.sync.dma_start(iix[:], iidx[:])
            xu = pool.tile([P, T, R], f32)
            xi = pool.tile([P, T, R], f32)
            prod = pool.tile([P, T, EMB], f32)
            r = pool.tile([P, T], f32)
            for t in range(T):
                nc.gpsimd.indirect_dma_start(
                    out=xu[:, t, :],
                    out_offset=None,
                    in_=utab[:],
                    in_offset=bass.IndirectOffsetOnAxis(ap=uix[:, t : t + 1], axis=0),
                )
                nc.gpsimd.indirect_dma_start(
                    out=xi[:, t, :],
                    out_offset=None,
                    in_=itab[:],
                    in_offset=bass.IndirectOffsetOnAxis(ap=iix[:, t : t + 1], axis=0),
                )
            # prod = xu_emb * xi_emb; r[:, t] = sum_k prod[:, t, k] + alphas
            nc.vector.tensor_tensor(
                out=prod[:],
                in0=xu[:, :, 0:EMB],
                in1=xi[:, :, 0:EMB],
                op=mybir.AluOpType.mult,
            )
            nc.vector.reduce_sum(r[:], prod[:], axis=mybir.AxisListType.X)
            nc.vector.tensor_add(
                out=r[:],
                in0=r[:],
                in1=xu[:, :, EMB : EMB + 1].rearrange("p t one -> p (t one)"),
            )
            nc.vector.tensor_add(
                out=r[:],
                in0=r[:],
                in1=xi[:, :, EMB : EMB + 1].rearrange("p t one -> p (t one)"),
            )
            nc.sync.dma_start(out[:], r[:])

    if finalize:
        nc.finalize()
        _nc_cache["nc"] = nc
    else:
        nc.compile()
    return nc


def _prep_tables(user_table, item_table, word_table, w_lin, V, bias_u, bias_i):
    """Weight-only preprocessing (reusable across batches)."""
    f32 = np.float32
    U_emb = np.concatenate([user_table, word_table[:N_USERS]], axis=1).astype(f32)
    I_emb = np.concatenate(
        [item_table, word_table[N_USERS : N_USERS + N_ITEMS]], axis=1
    ).astype(f32)
    V = np.asarray(V, f32)
    w_lin = np.asarray(w_lin, f32)
    V_U, V_I = V[:EMB], V[EMB:]
    s = (V * V).sum(axis=1)
    ZU = U_emb @ V_U
    alpha_u = (
        np.asarray(bias_u, f32)
        + U_emb @ w_lin[:EMB]
        + 0.5 * (ZU * ZU).sum(axis=1)
        - 0.5 * (U_emb * U_emb) @ s[:EMB]
        + 1.5
    )
    ZI = I_emb @ V_I
    alpha_i = (
        np.asarray(bias_i, f32)
        + I_emb @ w_lin[EMB:]
        + 0.5 * (ZI * ZI).sum(axis=1)
        - 0.5 * (I_emb * I_emb) @ s[EMB:]
        + 1.5
    )
    Uhat = U_emb @ (V_U @ V_I.T)
    utab = np.zeros((N_USERS, R), f32)
    utab[:, :EMB] = Uhat
    utab[:, EMB] = alpha_u
    itab = np.zeros((N_ITEMS, R), f32)
    itab[:, :EMB] = I_emb
    itab[:, EMB] = alpha_i
    return np.ascontiguousarray(utab), np.ascontiguousarray(itab)


def kernel(
    uid_batch,
    iid_batch,
    n_users,
    user_table,
    item_table,
    word_table,
    w_lin,
    V,
    bias_u,
    bias_i,
    _trace=False,
):
    uid = np.asarray(uid_batch).astype(np.int32)
    iid = np.asarray(iid_batch).astype(np.int32)
    utab, itab = _prep_tables(
        np.asarray(user_table, np.float32),
        np.asarray(item_table, np.float32),
        np.asarray(word_table, np.float32),
        w_lin,
        V,
        bias_u,
        bias_i,
    )

    nc = _build_nc()
    in_maps = []
    for c in range(N_CORES):
        us = uid[c * SHARD : (c + 1) * SHARD].reshape(P, T)
        is_ = iid[c * SHARD : (c + 1) * SHARD].reshape(P, T)
        in_maps.append(
            {
                "uidx": np.ascontiguousarray(us),
                "iidx": np.ascontiguousarray(is_),
                "utab": utab,
                "itab": itab,
            }
        )
    res = run_bass_kernel_spmd(
        nc, in_maps, core_ids=list(range(N_CORES)), trace=_trace
    )
    outs = [res.results[c]["out"].reshape(SHARD) for c in range(N_CORES)]
    full = np.concatenate(outs).astype(np.float32)
    if _trace:
        return full, res
    return full
